# revision 29
# baseline (speedup 1.0000x reference)
"""Trainium2 Bass kernel for nn_CrossAttention (single-head NxN attention + proj + InstanceNorm + residual).

Sharding: 8 cores = (batch b in 0..3) x (query-half h in 0..1).
Each core computes its half of the query tokens for one batch; the
InstanceNorm statistics (over the full 4096 tokens) are combined across
the core pair with a tiny AllGather.  A dummy warmup AllGather is issued
at kernel start (overlapping the prologue on the separate CC silicon) so
the real one at the end runs at its warm ~9us latency instead of ~45us.

Precision: matmul operands fp16 (all tensors here have tiny dynamic
range), accumulation fp32 in PSUM; x1 is loaded fp16 (residual rounding
~5e-4 rel, well within tolerance).  wq is pre-scaled by Cr^-0.5 and
wq/wk are pre-replicated 4x along the free dim on the host so q/k are
produced replicated across the four 32-partition groups in one matmul.

Self-contained: hardcodes shapes B=4, C=256, D=H=W=16 (N=4096), Cr=32.
"""

import numpy as np

import concourse.bass as bass
import concourse.mybir as mybir
import concourse.tile as tile
from concourse import bacc
from concourse.bass_utils import run_bass_kernel_spmd
from concourse.masks import make_identity

B, C, N, Cr = 4, 256, 4096, 32
NH = N // 2  # query tokens per core
EPS = 1e-5
SCALE = float(Cr) ** -0.5
FP32 = mybir.dt.float32
FP16 = mybir.dt.float16

N_CORES = 8
REPLICA_GROUPS = [[0, 1], [2, 3], [4, 5], [6, 7]]

IT = 256                   # i-tile width (query columns per superburst)
N_ITILES = NH // IT        # 8
JBLK = 128                 # j-block (rows per QK matmul output)
N_JBLK = N // JBLK         # 32
JB_PER_SB = 4              # j-blocks per superburst (4-way row-tiled QK)
SB_PER_IT = N_JBLK // JB_PER_SB  # 8
N_SB = N_ITILES * SB_PER_IT      # 64

AF = mybir.ActivationFunctionType
ALU = mybir.AluOpType

LAST_RESULTS = None  # BassKernelResults of the most recent run (for test harness)


def build_nc(use_collective=True):
    nc = bacc.Bacc("TRN2", num_devices=N_CORES, name="xattn",
                   target_bir_lowering=False)

    x1h_d = nc.dram_tensor("x1h", [C, NH], FP16, kind="ExternalInput").ap()
    x2b_d = nc.dram_tensor("x2b", [C, N], FP16, kind="ExternalInput").ap()
    wqT_d = nc.dram_tensor("wqT", [C, 128], FP16, kind="ExternalInput").ap()
    wkT_d = nc.dram_tensor("wkT", [C, 128], FP16, kind="ExternalInput").ap()
    wvT_d = nc.dram_tensor("wvT", [C, C], FP16, kind="ExternalInput").ap()
    wpT_d = nc.dram_tensor("wpT", [C, C], FP16, kind="ExternalInput").ap()
    out_d = nc.dram_tensor("out", [C, NH], FP32, kind="ExternalOutput").ap()

    with tile.TileContext(nc) as tc:
        build_body(tc, x1h_d, x2b_d, wqT_d, wkT_d, wvT_d, wpT_d, out_d,
                   use_collective)
    nc.compile()
    return nc


def build_body(tc, x1h_d, x2b_d, wqT_d, wkT_d, wvT_d, wpT_d, out_d,
               use_collective=True):
    nc = tc.nc
    from contextlib import ExitStack

    with ExitStack() as ctx:
        persist = ctx.enter_context(tc.tile_pool(name="persist", bufs=1))
        sm = ctx.enter_context(tc.tile_pool(name="sm", bufs=4))
        avcp = ctx.enter_context(tc.tile_pool(name="avcp", bufs=4))
        ptp = ctx.enter_context(tc.tile_pool(name="ptp", bufs=3))
        qkp = ctx.enter_context(tc.tile_pool(name="qkp", bufs=2, space="PSUM"))
        avp = ctx.enter_context(tc.tile_pool(name="avp", bufs=2, space="PSUM"))
        # 1KB slots shared by the prologue vp tiles and the epilogue tp/pj
        # tiles (they never overlap in time): 2 slots = 1 PSUM bank
        epi = ctx.enter_context(tc.tile_pool(name="epi", bufs=2, space="PSUM"))
        dramp = ctx.enter_context(tc.tile_pool(name="dramp", bufs=1, space="DRAM"))

        # ---- warmup collective: absorbs the ~45us first-collective cost on
        # the CC path while the PE/ACT prologue runs -----------------------
        if use_collective:
            warm_sb = persist.tile([128, 4], FP32, tag="warm", name="warm_sb")
            nc.vector.memset(warm_sb, 0.0)
            warm_in = dramp.tile([128, 4], FP32, tag="warm_i", name="warm_in")
            warm_out = dramp.tile([2, 128, 4], FP32, tag="warm_o", name="warm_out")
            nc.sync.dma_start(warm_in, warm_sb)
            nc.gpsimd.collective_compute(
                "AllGather", ALU.bypass, replica_groups=REPLICA_GROUPS,
                ins=[warm_in.opt()], outs=[warm_out.opt()])

        # ---- constants -------------------------------------------------
        eps_sb = persist.tile([128, 1], FP32, tag="eps", name="eps_sb")
        nc.vector.memset(eps_sb, EPS)
        ident = persist.tile([128, 128], FP16, tag="ident", name="ident")
        make_identity(nc, ident)

        # ---- input DMAs (x2 first: v/k matmuls start earliest) ---------
        wv_sb = [persist.tile([128, C], FP16, tag=f"wv{cc}", name=f"wv_sb{cc}")
                 for cc in range(2)]
        wk_sb = [persist.tile([128, 128], FP16, tag=f"wk{cc}", name=f"wk_sb{cc}")
                 for cc in range(2)]
        wq_sb = [persist.tile([128, 128], FP16, tag=f"wq{cc}", name=f"wq_sb{cc}")
                 for cc in range(2)]
        wp_sb = [persist.tile([128, C], FP16, tag=f"wp{cc}", name=f"wp_sb{cc}")
                 for cc in range(2)]
        # weights on the scalar queue, x2 halves split over sync/vector
        # queues, x1 on the tensor queue: four DMA rings run concurrently
        # instead of serializing ~3.4MB through one ring.
        for cc in range(2):
            sl = slice(128 * cc, 128 * (cc + 1))
            nc.scalar.dma_start(wv_sb[cc], wvT_d[sl, :])
            nc.scalar.dma_start(wk_sb[cc], wkT_d[sl, :])
            nc.scalar.dma_start(wq_sb[cc], wqT_d[sl, :])
            nc.scalar.dma_start(wp_sb[cc], wpT_d[sl, :])

        x2_sb = [persist.tile([128, N], FP16, tag=f"x2_{cc}", name=f"x2_sb{cc}")
                 for cc in range(2)]
        x1_sb = [persist.tile([128, NH], FP16, tag=f"x1_{cc}", name=f"x1_sb{cc}")
                 for cc in range(2)]
        x2q = [nc.sync, nc.gpsimd]
        for ch in range(8):
            sl = slice(512 * ch, 512 * (ch + 1))
            for cc in range(2):
                x2q[cc].dma_start(x2_sb[cc][:, sl],
                                  x2b_d[128 * cc:128 * (cc + 1), sl])
            if ch < 4:  # x1 quarters right after the matching x2 slices
                slh = slice(512 * ch, 512 * (ch + 1))
                for cc in range(2):
                    nc.scalar.dma_start(x1_sb[cc][:, slh],
                                        x1h_d[128 * cc:128 * (cc + 1), slh])

        # ---- prologue: vt / k_rep / q_rep ------------------------------
        # vt = x2^T @ wv^T -> [j, c] fp16 with ones column (softmax denom)
        vt = persist.tile([128, N_JBLK, C + 1], FP16, tag="vt", name="vt")
        nc.vector.memset(vt[:, :, C:C + 1], 1.0)
        k_rep = persist.tile([128, N], FP16, tag="krep", name="k_rep")
        q_rep = persist.tile([128, NH], FP16, tag="qrep", name="q_rep")

        def emit_v(jblk):
            vp = epi.tile([128, C], FP32, tag="epi", name=f"vp{jblk}")
            for cc in range(2):
                nc.tensor.matmul(
                    vp, lhsT=x2_sb[cc][:, 128 * jblk:128 * (jblk + 1)],
                    rhs=wv_sb[cc], start=(cc == 0), stop=(cc == 1))
            if jblk % 2 == 0:
                nc.vector.tensor_copy(vt[:, jblk, 0:C], vp)
            else:
                nc.scalar.copy(vt[:, jblk, 0:C], vp)

        def emit_k(jt):
            kp = qkp.tile([128, 512], FP32, tag="qk", name=f"kp{jt}")
            for cc in range(2):
                nc.tensor.matmul(
                    kp, lhsT=wk_sb[cc],
                    rhs=x2_sb[cc][:, 512 * jt:512 * (jt + 1)],
                    start=(cc == 0), stop=(cc == 1))
            nc.vector.tensor_copy(k_rep[:, 512 * jt:512 * (jt + 1)], kp)

        def emit_q(qt):
            qp = qkp.tile([128, 512], FP32, tag="qk", name=f"qp{qt}")
            for cc in range(2):
                nc.tensor.matmul(
                    qp, lhsT=wq_sb[cc],
                    rhs=x1_sb[cc][:, 512 * qt:512 * (qt + 1)],
                    start=(cc == 0), stop=(cc == 1))
            nc.vector.tensor_copy(q_rep[:, 512 * qt:512 * (qt + 1)], qp)

        def emit_unit(u):
            # prologue unit u builds exactly what superburst sb_idx=u of the
            # first i-tile consumes: vt[4u..4u+3], k_rep[:, 512u:512(u+1)].
            # Units are woven into the first i-tile's superbursts so the main
            # loop starts as soon as the first x2/x1 slices arrive instead of
            # stalling behind the whole 3.4MB input DMA.
            for jblk in range(4 * u, 4 * (u + 1)):
                emit_v(jblk)
            emit_k(u)
            if u < 4:
                emit_q(u)

        # ---- persistent attention outputs ------------------------------
        proj_sb = [persist.tile([128, NH], FP32, tag=f"proj{ob}", name=f"proj_sb{ob}")
                   for ob in range(2)]
        stats_sb = [persist.tile([128, N_ITILES, 6], FP32, tag=f"stats{ob}",
                                 name=f"stats_sb{ob}") for ob in range(2)]

        # ---- main loop: 64 superbursts of (4-way row-tiled QK, exp, AV) -
        def emit_qk(s):
            it, sb = s // SB_PER_IT, s % SB_PER_IT
            isl = slice(IT * it, IT * (it + 1))
            qk = qkp.tile([128, IT * JB_PER_SB], FP32, tag="qk", name=f"qk{s}")
            # issue order t0,t2,t1,t3: concurrent pairs (t0,t2),(t1,t3) land in
            # different PSUM banks; same-bank pairs share a row group so the
            # hardware serializes them (same-bank concurrent drains hang).
            for t in (0, 2, 1, 3):
                jblk = sb * JB_PER_SB + t
                g = t // 2
                nc.tensor.matmul(
                    qk[:, IT * t:IT * (t + 1)],
                    lhsT=k_rep[32 * g:32 * (g + 1),
                               JBLK * jblk:JBLK * (jblk + 1)],
                    rhs=q_rep[32 * g:32 * (g + 1), isl],
                    start=True, stop=True, tile_position=(32 * g, 0))
            pt = ptp.tile([128, IT * JB_PER_SB], FP16, tag="pt", name=f"pt{s}")
            nc.scalar.activation(out=pt, in_=qk, func=AF.Exp)
            return pt

        def emit_av(s, pt, av_t):
            it, sb = s // SB_PER_IT, s % SB_PER_IT
            for t in range(JB_PER_SB):
                jblk = sb * JB_PER_SB + t
                for ib in range(IT // 128):
                    nc.tensor.matmul(
                        av_t[ib][:, 0:C + 1],
                        lhsT=pt[:, IT * t + 128 * ib:IT * t + 128 * (ib + 1)],
                        rhs=vt[:, jblk, :],
                        start=(sb == 0 and t == 0),
                        stop=(sb == SB_PER_IT - 1 and t == JB_PER_SB - 1))

        avc = {}

        def epi_chunk_a(it, ib, av_t):
            rden = sm.tile([128, 1], FP32, tag="rden", name=f"rden{it}_{ib}")
            nc.vector.reciprocal(rden, av_t[ib][:, C:C + 1])
            avn = sm.tile([128, C], FP16, tag="avn", name=f"avn{it}_{ib}")
            nc.vector.tensor_scalar_mul(avn, in0=av_t[ib][:, 0:C], scalar1=rden)
            tp = epi.tile([128, C], FP16, tag="epi", name=f"tp{it}_{ib}")
            nc.tensor.transpose(tp[:, 0:128], avn[:, 0:128], ident)
            nc.tensor.transpose(tp[:, 128:256], avn[:, 128:256], ident)
            for cc in range(2):
                nc.vector.tensor_copy(avc[(it, cc)][:, 128 * ib:128 * (ib + 1)],
                                      tp[:, 128 * cc:128 * (cc + 1)])

        def epi_chunk_b(it, ob):
            isl = slice(IT * it, IT * (it + 1))
            pj = epi.tile([128, IT], FP32, tag="epi", name=f"pj{it}_{ob}")
            for cc in range(2):
                nc.tensor.matmul(
                    pj, lhsT=wp_sb[cc][:, 128 * ob:128 * (ob + 1)],
                    rhs=avc[(it, cc)], start=(cc == 0), stop=(cc == 1))
            nc.vector.tensor_copy(proj_sb[ob][:, isl], pj)
            nc.vector.bn_stats(stats_sb[ob][:, it, :], proj_sb[ob][:, isl])

        # Epilogue chunks drain one per superburst; the chunk_a reads of an
        # i-tile's av_t PSUM are always emitted before the avp pool slot is
        # handed to a later i-tile (chunk_a(it,0) drains at sb==7 of it,
        # chunk_a(it,1) at the very start of the next superburst).
        pending = []
        emit_unit(0)
        emit_unit(1)
        pts = {0: emit_qk(0), 1: emit_qk(1)}
        av_t = None
        for s in range(N_SB):
            it, sb = s // SB_PER_IT, s % SB_PER_IT
            if pending:
                pending.pop(0)()
            if sb == 0:
                # [128, 512] fp32 tiles = exactly one PSUM bank, so every avp
                # slot is bank-aligned (accumulation must not straddle banks);
                # only cols 0..C hold data (C..C+1 is the ones-column denom).
                av_t = [avp.tile([128, 512], FP32, tag="av", name=f"av{it}_{ib}")
                        for ib in range(IT // 128)]
            if s + 2 < SB_PER_IT:
                emit_unit(s + 2)  # weave remaining prologue through i-tile 0
            if s + 2 < N_SB:
                pts[s + 2] = emit_qk(s + 2)
            emit_av(s, pts.pop(s), av_t)
            if sb == SB_PER_IT - 1:
                for cc in range(2):
                    avc[(it, cc)] = avcp.tile([128, IT], FP16, tag=f"avc{cc}",
                                              name=f"avc{it}_{cc}")
                at = av_t
                pending.extend([
                    (lambda it=it, at=at: epi_chunk_a(it, 0, at)),
                    (lambda it=it, at=at: epi_chunk_a(it, 1, at)),
                    (lambda it=it: epi_chunk_b(it, 0)),
                    (lambda it=it: epi_chunk_b(it, 1)),
                ])
                pending.pop(0)()
        while pending:
            pending.pop(0)()

        # ---- cross-core InstanceNorm stats -----------------------------
        ccin = persist.tile([128, 4], FP32, tag="ccin", name="ccin")
        for ob in range(2):
            mv = sm.tile([128, 2], FP32, tag="mv", name=f"mv{ob}")
            nc.vector.bn_aggr(out=mv, in_=stats_sb[ob])
            nc.vector.tensor_copy(ccin[:, 2 * ob:2 * (ob + 1)], mv)

        cc8 = persist.tile([128, 2, 2, 2], FP32, tag="cc8", name="cc8")
        if use_collective:
            ccin_dr = dramp.tile([128, 4], FP32, tag="ccin_d", name="ccin_dr")
            ccout_dr = dramp.tile([2, 128, 4], FP32, tag="ccout_d", name="ccout_dr")
            nc.sync.dma_start(ccin_dr, ccin)
            nc.gpsimd.collective_compute(
                "AllGather", ALU.bypass, replica_groups=REPLICA_GROUPS,
                ins=[ccin_dr.opt()], outs=[ccout_dr.opt()])
            # one strided DMA: [r, p, c] -> [p, r, ob, {m,v}]
            nc.sync.dma_start(cc8, ccout_dr.rearrange("r p c -> p r c")
                              .rearrange("p r (o s) -> p r o s", o=2))
        else:
            nc.vector.tensor_copy(cc8[:, 0], ccin.rearrange("p (o s) -> p o s", o=2))
            nc.vector.tensor_copy(cc8[:, 1], ccin.rearrange("p (o s) -> p o s", o=2))

        # combined pair stats (both output halves at once):
        # mean = (mA+mB)/2 ; var = (vA+vB)/2 + (mA-mB)^2/4 ; rstd = rsqrt(var+eps)
        tot = sm.tile([128, 2, 2], FP32, tag="tot", name="tot")
        nc.vector.tensor_add(tot, cc8[:, 0], cc8[:, 1])
        dif = sm.tile([128, 2, 2], FP32, tag="dif", name="dif")
        nc.vector.tensor_sub(dif, cc8[:, 0], cc8[:, 1])
        mean2 = persist.tile([128, 2], FP32, tag="mean2", name="mean2")
        nc.vector.tensor_scalar_mul(mean2, in0=tot[:, :, 0], scalar1=0.5)
        d2 = sm.tile([128, 2], FP32, tag="d2", name="d2")
        nc.vector.tensor_mul(d2, dif[:, :, 0], dif[:, :, 0])
        var2 = sm.tile([128, 2], FP32, tag="var2", name="var2")
        nc.vector.tensor_scalar(out=var2, in0=tot[:, :, 1], scalar1=0.5,
                                scalar2=None, op0=ALU.mult)
        nc.vector.tensor_scalar(out=d2, in0=d2, scalar1=0.25, scalar2=None,
                                op0=ALU.mult)
        nc.vector.tensor_add(var2, var2, d2)
        rstd2 = persist.tile([128, 2], FP32, tag="rstd2", name="rstd2")
        nc.scalar.activation(out=rstd2, in_=var2, func=AF.Sqrt, bias=eps_sb,
                             scale=1.0)
        nc.vector.reciprocal(rstd2, rstd2)
        negmr = persist.tile([128, 2], FP32, tag="negmr", name="negmr")
        nc.vector.tensor_mul(negmr, mean2, rstd2)
        nc.vector.tensor_scalar_mul(negmr, in0=negmr, scalar1=-1.0)

        # ---- final norm + residual + store -----------------------------
        # (proj - mean)*rstd as an ACT affine (per-partition scale/bias),
        # + x1 on DVE, stores overlapping.  GpSimd is useless here: its
        # tensor_scalar on [128,512] measures ~8us and starves the DVE.
        stq = [nc.sync, nc.gpsimd]
        for ob in range(2):
            rstd = rstd2[:, ob:ob + 1]
            nmr = negmr[:, ob:ob + 1]
            for ch in range(4):
                sl = slice(512 * ch, 512 * (ch + 1))
                nc.scalar.activation(
                    out=proj_sb[ob][:, sl], in_=proj_sb[ob][:, sl],
                    func=AF.Identity, bias=nmr, scale=rstd)
                nc.vector.tensor_add(proj_sb[ob][:, sl], proj_sb[ob][:, sl],
                                     x1_sb[ob][:, sl])
                stq[(ob * 4 + ch) % 2].dma_start(
                    out_d[128 * ob:128 * (ob + 1), sl], proj_sb[ob][:, sl])


_nc_cache = None


def _get_nc():
    global _nc_cache
    if _nc_cache is None:
        import os
        _nc_cache = build_nc(
            use_collective=not os.environ.get("XATTN_NO_COLLECTIVE"))
    return _nc_cache


def make_in_maps(x1, x2, wq, wk, wv, wp):
    x1f = np.asarray(x1, np.float32).reshape(B, C, N).astype(np.float16)
    x2f = np.asarray(x2, np.float32).reshape(B, C, N).astype(np.float16)
    wqT = np.ascontiguousarray(
        np.tile((np.asarray(wq, np.float32) * SCALE).T, (1, 4)).astype(np.float16))
    wkT = np.ascontiguousarray(
        np.tile(np.asarray(wk, np.float32).T, (1, 4)).astype(np.float16))
    wvT = np.ascontiguousarray((np.asarray(wv, np.float32).T).astype(np.float16))
    wpT = np.ascontiguousarray((np.asarray(wp, np.float32).T).astype(np.float16))
    in_maps = []
    for core in range(N_CORES):
        b, h = core // 2, core % 2
        in_maps.append({
            "x1h": np.ascontiguousarray(x1f[b, :, h * NH:(h + 1) * NH]),
            "x2b": np.ascontiguousarray(x2f[b]),
            "wqT": wqT, "wkT": wkT, "wvT": wvT, "wpT": wpT,
        })
    return in_maps


def assemble_out(results):
    out = np.empty((B, C, N), np.float32)
    for core in range(N_CORES):
        b, h = core // 2, core % 2
        out[b, :, h * NH:(h + 1) * NH] = results[core]["out"]
    return out.reshape(B, C, 16, 16, 16)


def kernel(**inputs):
    global LAST_RESULTS
    in_maps = make_in_maps(inputs["x1"], inputs["x2"], inputs["wq"],
                           inputs["wk"], inputs["wv"], inputs["wp"])
    res = run_bass_kernel_spmd(_get_nc(), in_maps, core_ids=list(range(N_CORES)))
    LAST_RESULTS = res
    return assemble_out(res.results)
